# revision 26
# baseline (speedup 1.0000x reference)
"""Distributed exact inner-product top-k (brute-force kNN) on 8 TRN2 NeuronCores.

Sharding: codebook W is split row-wise into 8 shards of 25000 (one per core,
padded to 25088 = 24 tiles x 1024 + 512 with zero columns); x is replicated.
Host pre-transposes both so the contraction dim (128) lands on SBUF partitions.

Device kernel (SPMD, no collectives), per 128-row group and 1024-col tile:
  - 2 x 512-wide bf16 matmuls -> PSUM f32 scores (512 = ISA cap per matmul;
    4-deep PSUM tile pipeline hides the drain-chain sync latency)
  - the tile is drained half by ACT (fp16 copy of the lo half, issued right
    after the first matmul) and half by DVE (tensor_tensor max pairing the
    PSUM hi half against that copy -- two PSUM operands are illegal, and
    this one pass is both the PSUM drain and the pair-reduction); the even
    split keeps ACT (0.833ns/elem) and DVE (1.04ns/elem on a PSUM operand)
    within ~5% of each other, and both run ~95% busy
  - the resulting window maxima (window = 2 cols: {c, c+512} within the
    tile) go straight to DRAM as [1024, 12544] fp16 per core, on the idle
    gpsimd engine's DGE ring (alternating with sync) so output transfers
    never queue behind each other.  There is no max8/find_index8 (1
    elem/cycle + a second full scan dominated the original kernel) and no
    deeper on-device reduction (DMA bandwidth is cheaper than DVE cycles
    at this balance point).

Host merge: per row, t_hat = 128th-largest stored window max; every window
with stored >= t_hat - B is recomputed exactly in f64.  A window hiding a
true top-128 element necessarily has stored max >= t_hat - 2*eps where eps
bounds |stored - exact| (bf16 matmul noise + fp16 rounding, measured well
under 0.3); B = 1.0 covers it with margin.  The measured eps is verified
against the recomputed windows every run; if it nears B/2 the selection is
redone with a wider B from the same stored values, and any bad row falls
back to full exact recompute.  Final top-128 ordered like jax.lax.top_k
(value desc, index asc; ~12 of 131072 entries differ from the reference
where f32 score ties rank differently -- same as the previous kernel).
"""

import numpy as np

B = 1024
D = 128
VOCAB = 200000
NCORES = 8
VSHARD = VOCAB // NCORES   # 25000
NFT = 24                   # full 1024-col tiles per core
TILE = 1024
TAIL = 512                 # tail tile cols
COLS = NFT * TILE + TAIL   # 25088 (padded shard width)
WIN = 2                    # columns per window
NWIN_FT = 512              # windows per full tile
NWIN_TAIL = 256
NWIN = NFT * NWIN_FT + NWIN_TAIL      # 12544 windows per core per row
NWIN_ALL = NCORES * NWIN              # 100352 windows per row
TOPK = 128
MMW = 512

# Tiles whose PSUM is fully drained by ACT (DVE then pair-maxes in fp16)
# instead of the even ACT/DVE split.  Empirically the plain even split
# (empty set) balances best; kept as a tuning knob.
FULL_ACT_TILES = frozenset()

# |stored fp8 window max - exact f64 window max|: bf16 matmul noise
# (measured < 0.2 on this data) + fp8e4 storage rounding (<= 4 for values
# up to 128).  B must exceed twice that; verified against the recomputed
# windows every run and widened automatically if ever violated.
B_SLACK = 10.0

LAST_RESULTS = None  # BassKernelResults of the most recent run (for profiling)
_CACHED_NC = None
_WINCOLS = None      # [NWIN_ALL, WIN] int64 global col per window, -1 invalid


def build_kernel():
    import concourse.bass as bass  # noqa: F401
    import concourse.tile as tile
    from concourse import bacc, mybir

    F32 = mybir.dt.float32
    BF16 = mybir.dt.bfloat16
    FP16 = mybir.dt.float16
    FP8 = mybir.dt.float8e4
    MAX = mybir.AluOpType.max

    nc = bacc.Bacc("TRN2", target_bir_lowering=False, debug=False)
    wt_d = nc.dram_tensor("wt", [D, COLS], BF16, kind="ExternalInput")
    xt_d = nc.dram_tensor("xt", [D, B], BF16, kind="ExternalInput")
    wmax_d = nc.dram_tensor("out_wmax", [B, NWIN], FP8, kind="ExternalOutput")

    L1W = NFT * 512 + TAIL // 2  # 12544

    with tile.TileContext(nc) as tc:
        with (
            tc.tile_pool(name="wt", bufs=1) as wt_pool,
            tc.tile_pool(name="xt", bufs=1) as xt_pool,
            tc.tile_pool(name="psum", bufs=4, space="PSUM") as psum_pool,
            tc.tile_pool(name="hi", bufs=6) as hi_pool,
            tc.tile_pool(name="hf", bufs=4) as hf_pool,
            tc.tile_pool(name="l1", bufs=3) as l1_pool,
            tc.tile_pool(name="tree", bufs=1) as tree_pool,
            tc.tile_pool(name="wm", bufs=2) as wm_pool,
        ):
            wt_sb = wt_pool.tile([D, COLS], BF16)
            xt_sb = xt_pool.tile([D, B], BF16)
            # xt first so group 0's stationary is ready; W lands in tile order
            # so group 0's tile t can start as soon as slice t is in.
            nc.sync.dma_start(xt_sb[:], xt_d[:])
            for t in range(0, NFT + 1, 2):
                c0 = t * TILE
                c1 = min(COLS, c0 + 2 * TILE)
                nc.sync.dma_start(wt_sb[:, c0:c1], wt_d[:, c0:c1])

            # Software-pipelined: group g-1's output DMA is issued in the
            # middle of group g so it never blocks the drain chain.
            NG = B // 128
            # Last group's l1 is split into 6 tiles so its output DMA can
            # start before the group finishes (no long drain at the end).
            SEGW = 8 * 512
            SEGB = list(range(0, L1W, SEGW))
            pending = None  # (l1_tile, group) of the previous group
            for g in range(NG):
                if g < NG - 1:
                    l1 = l1_pool.tile([128, L1W], FP8)
                    segs = [(l1, 0)]
                else:
                    segs = []
                    for i, b0 in enumerate(SEGB):
                        w = min(SEGW, L1W - b0)
                        st = l1_pool.tile([128, w], FP8, tag=f"l1s{i}")
                        segs.append((st, b0))

                def lslice(o0, o1):
                    for seg, base in reversed(segs):
                        if o0 >= base:
                            return seg[:, o0 - base:o1 - base]
                    raise AssertionError

                for t in range(NFT + 1):
                    ps = psum_pool.tile([128, TILE], F32)
                    if t < NFT:
                        for m in range(TILE // MMW):
                            c0 = t * TILE + m * MMW
                            nc.tensor.matmul(
                                ps[:, m * MMW:(m + 1) * MMW],
                                xt_sb[:, g * 128:(g + 1) * 128],
                                wt_sb[:, c0:c0 + MMW],
                                start=True, stop=True,
                            )
                        lo = lslice(t * 512, (t + 1) * 512)
                        if t in FULL_ACT_TILES and g % 2 == 0:
                            h = hf_pool.tile([128, 1024], FP16, tag="hf")
                            nc.scalar.copy(h[:], ps[:])
                            nc.vector.tensor_tensor(
                                lo, h[:, 0:512], h[:, 512:1024], MAX
                            )
                        else:
                            h = hi_pool.tile([128, 512], FP16, tag="hi")
                            nc.scalar.copy(h[:], ps[:, 0:512])
                            nc.vector.tensor_tensor(
                                lo, ps[:, 512:1024], h[:], MAX
                            )
                    else:
                        # tail tile: 1 matmul of 512, split drain
                        nc.tensor.matmul(
                            ps[:, 0:TAIL],
                            xt_sb[:, g * 128:(g + 1) * 128],
                            wt_sb[:, NFT * TILE:COLS],
                            start=True, stop=True,
                        )
                        ht = hi_pool.tile([128, 512], FP16, tag="hi")
                        nc.scalar.copy(ht[:, 0:256], ps[:, 0:256])
                        nc.vector.tensor_tensor(
                            lslice(NFT * 512, L1W), ps[:, 256:512],
                            ht[:, 0:256], MAX,
                        )
                    if pending is not None and t == 8:
                        pl1, pg = pending
                        eng = nc.gpsimd if pg % 2 == 0 else nc.sync
                        eng.dma_start(
                            wmax_d[pg * 128:(pg + 1) * 128, :], pl1[:]
                        )
                        pending = None
                    if (g == NG - 1 and t in (8, 16)
                            and t // 8 <= len(SEGB) - 2):
                        seg, base = segs[t // 8 - 1]
                        nc.gpsimd.dma_start(
                            wmax_d[g * 128:(g + 1) * 128,
                                   base:base + seg.shape[1]],
                            seg[:],
                        )
                if g < NG - 1:
                    pending = (l1, g)
            g = NG - 1
            for i, (seg, base) in enumerate(segs[len(SEGB) - 2:]):
                eng = nc.gpsimd if i % 2 == 0 else nc.sync
                eng.dma_start(
                    wmax_d[g * 128:(g + 1) * 128, base:base + seg.shape[1]],
                    seg[:],
                )
    nc.compile()
    return nc


def _wincols():
    """[NWIN_ALL, WIN] global column per (core, tile, j) window; -1 invalid.

    Full tile t, window j in [0,512): cols t*1024 + j + 512*k, k in {0,1}.
    Tail tile, window j in [0,256):   cols 24576 + j + 256*k,  k in {0,1}.
    """
    global _WINCOLS
    if _WINCOLS is None:
        k = np.arange(WIN)
        full = (
            np.arange(NFT)[:, None, None] * TILE
            + np.arange(NWIN_FT)[None, :, None]
            + 512 * k[None, None, :]
        ).reshape(NFT * NWIN_FT, WIN)
        tail = (
            NFT * TILE + np.arange(NWIN_TAIL)[:, None] + 256 * k[None, :]
        )
        local = np.concatenate([full, tail], axis=0)  # [NWIN, WIN]
        cols = (
            np.arange(NCORES)[:, None, None] * VSHARD + local[None]
        ).reshape(NWIN_ALL, WIN)
        invalid = np.broadcast_to(
            local[None] >= VSHARD, (NCORES, NWIN, WIN)
        ).reshape(NWIN_ALL, WIN)
        cols = cols.copy()
        cols[invalid] = -1
        _WINCOLS = cols.astype(np.int64)
    return _WINCOLS


def _merge(x64, W64, stored, Bw):
    """Exact top-K from device window maxima.  Returns (idx, eps, bad_rows)."""
    wincols = _wincols()
    vals = stored  # [B, NWIN_ALL] f32
    thr = np.partition(vals, NWIN_ALL - TOPK, axis=1)[:, NWIN_ALL - TOPK]
    sel = vals >= (thr[:, None] - Bw)

    out = np.empty((B, TOPK), dtype=np.int64)
    eps = 0.0
    bad_rows = []
    STEP = 64
    for r0 in range(0, B, STEP):
        r1 = r0 + STEP
        sblk = sel[r0:r1]
        maxw = int(sblk.sum(axis=1).max())
        wid = np.full((STEP, maxw), -1, dtype=np.int64)
        for i in range(STEP):
            w = np.flatnonzero(sblk[i])
            wid[i, :len(w)] = w
        cols = np.where(
            wid[:, :, None] >= 0, wincols[wid], -1
        ).reshape(STEP, maxw * WIN)
        valid = cols >= 0
        gW = W64[np.where(valid, cols, 0)]
        exact = np.einsum("bjd,bd->bj", gW, x64[r0:r1])
        exact[~valid] = -np.inf

        ew = exact.reshape(STEP, maxw, WIN).max(axis=2)
        wv = wid >= 0
        dv = np.take_along_axis(vals[r0:r1], np.clip(wid, 0, None), axis=1)
        fin = wv & np.isfinite(ew)
        if fin.any():
            eps = max(eps, float(np.abs(np.where(fin, dv - ew, 0.0)).max()))

        order = np.lexsort((np.where(valid, cols, 2**62), -exact), axis=1)
        top = order[:, :TOPK]
        tv = np.take_along_axis(exact, top, axis=1)
        if not np.isfinite(tv).all():
            bad_rows.extend(r0 + np.flatnonzero(~np.isfinite(tv).all(axis=1)))
        out[r0:r1] = np.take_along_axis(cols, top, axis=1)
    return out, eps, bad_rows


def kernel(x: np.ndarray, W: np.ndarray, topk) -> np.ndarray:
    global LAST_RESULTS, _CACHED_NC
    import os

    import ml_dtypes

    from concourse.bass_utils import run_bass_kernel_spmd

    assert x.shape == (B, D) and W.shape == (VOCAB, D)
    assert int(topk) == TOPK
    x = np.ascontiguousarray(np.asarray(x, dtype=np.float32))
    W = np.ascontiguousarray(np.asarray(W, dtype=np.float32))

    if _CACHED_NC is None:
        _CACHED_NC = build_kernel()
    nc = _CACHED_NC

    xt = np.ascontiguousarray(x.T).astype(ml_dtypes.bfloat16)
    in_maps = []
    for i in range(NCORES):
        sh = np.zeros((D, COLS), dtype=ml_dtypes.bfloat16)
        sh[:, :VSHARD] = W[i * VSHARD:(i + 1) * VSHARD].T.astype(
            ml_dtypes.bfloat16
        )
        in_maps.append({"wt": sh, "xt": xt})

    LAST_RESULTS = run_bass_kernel_spmd(
        nc,
        in_maps,
        core_ids=list(range(NCORES)),
        trace=bool(int(os.environ.get("KERNEL_TRACE", "0"))),
    )
    results = LAST_RESULTS.results

    stored = np.concatenate(
        [np.asarray(results[i]["out_wmax"])
         .view(ml_dtypes.float8_e4m3fn).astype(np.float32)
         for i in range(NCORES)],
        axis=1,
    )  # [B, NWIN_ALL]

    x64 = x.astype(np.float64)
    W64 = W.astype(np.float64)

    Bw = B_SLACK
    for _ in range(3):
        out, eps, bad_rows = _merge(x64, W64, stored, Bw)
        if 2.0 * eps + 0.15 <= Bw and not bad_rows:
            break
        Bw = max(2.0 * (2.0 * eps + 0.15), 2.0 * Bw)
    else:
        bad_rows = list(range(B))

    for r in set(int(r) for r in bad_rows):
        s = x64[r] @ W64.T
        out[r] = np.lexsort((np.arange(VOCAB), -s))[:TOPK]

    return out.astype(np.int32)


# revision 27
# speedup vs baseline: 1.1784x; 1.1784x over previous
"""Distributed exact inner-product top-k (brute-force kNN) on 8 TRN2 NeuronCores.

Sharding: codebook W is split row-wise into 8 shards of 25000 (one per core,
padded to 25088 = 24 tiles x 1024 + 512 with zero columns); x is replicated.
Host pre-transposes both so the contraction dim (128) lands on SBUF partitions.

Device kernel (SPMD, no collectives), per 128-row group and 1024-col tile:
  - 2 x 512-wide bf16 matmuls -> PSUM f32 scores (512 = ISA cap per matmul;
    4-deep PSUM tile pipeline hides the drain-chain sync latency)
  - the tile is drained half by ACT (fp16 copy of the lo half, issued right
    after the first matmul) and half by DVE (tensor_tensor max pairing the
    PSUM hi half against that copy -- two PSUM operands are illegal, and
    this one pass is both the PSUM drain and the pair-reduction); the even
    split keeps ACT (0.833ns/elem) and DVE (1.04ns/elem on a PSUM operand)
    within ~5% of each other, and both run ~95% busy
  - the resulting window maxima (window = 2 cols: {c, c+512} within the
    tile) go straight to DRAM as [1024, 12544] fp16 per core, on the idle
    gpsimd engine's DGE ring (alternating with sync) so output transfers
    never queue behind each other.  There is no max8/find_index8 (1
    elem/cycle + a second full scan dominated the original kernel) and no
    deeper on-device reduction (DMA bandwidth is cheaper than DVE cycles
    at this balance point).

Host merge: per row, t_hat = 128th-largest stored window max; every window
with stored >= t_hat - B is recomputed exactly in f64.  A window hiding a
true top-128 element necessarily has stored max >= t_hat - 2*eps where eps
bounds |stored - exact| (bf16 matmul noise + fp16 rounding, measured well
under 0.3); B = 1.0 covers it with margin.  The measured eps is verified
against the recomputed windows every run; if it nears B/2 the selection is
redone with a wider B from the same stored values, and any bad row falls
back to full exact recompute.  Final top-128 ordered like jax.lax.top_k
(value desc, index asc; ~12 of 131072 entries differ from the reference
where f32 score ties rank differently -- same as the previous kernel).
"""

import numpy as np

B = 1024
D = 128
VOCAB = 200000
NCORES = 8
VSHARD = VOCAB // NCORES   # 25000
NFT = 24                   # full 1024-col tiles per core
TILE = 1024
TAIL = 512                 # tail tile cols
COLS = NFT * TILE + TAIL   # 25088 (padded shard width)
WIN = 2                    # columns per window
NWIN_FT = 512              # windows per full tile
NWIN_TAIL = 256
NWIN = NFT * NWIN_FT + NWIN_TAIL      # 12544 windows per core per row
NWIN_ALL = NCORES * NWIN              # 100352 windows per row
TOPK = 128
MMW = 512

# Tiles whose PSUM is fully drained by ACT (DVE then pair-maxes in fp16)
# instead of the even ACT/DVE split.  Empirically the plain even split
# (empty set) balances best; kept as a tuning knob.
FULL_ACT_TILES = frozenset()

# |stored fp8 window max - exact f64 window max|: bf16 matmul noise
# (measured < 0.2 on this data) + fp8e4 storage rounding (<= 4 for values
# up to 128).  B must exceed twice that; verified against the recomputed
# windows every run and widened automatically if ever violated.
B_SLACK = 10.0

LAST_RESULTS = None  # BassKernelResults of the most recent run (for profiling)
_CACHED_NC = None
_WINCOLS = None      # [NWIN_ALL, WIN] int64 global col per window, -1 invalid


def build_kernel():
    import concourse.bass as bass  # noqa: F401
    import concourse.tile as tile
    from concourse import bacc, mybir

    F32 = mybir.dt.float32
    BF16 = mybir.dt.bfloat16
    FP16 = mybir.dt.float16
    FP8 = mybir.dt.float8e4
    MAX = mybir.AluOpType.max

    nc = bacc.Bacc("TRN2", target_bir_lowering=False, debug=False)
    wt_d = nc.dram_tensor("wt", [D, COLS], BF16, kind="ExternalInput")
    xt_d = nc.dram_tensor("xt", [D, B], BF16, kind="ExternalInput")
    wmax_d = nc.dram_tensor("out_wmax", [B, NWIN], FP8, kind="ExternalOutput")

    L1W = NFT * 512 + TAIL // 2  # 12544

    with tile.TileContext(nc) as tc:
        with (
            tc.tile_pool(name="wt", bufs=1) as wt_pool,
            tc.tile_pool(name="xt", bufs=1) as xt_pool,
            tc.tile_pool(name="psum", bufs=4, space="PSUM") as psum_pool,
            tc.tile_pool(name="hi", bufs=6) as hi_pool,
            tc.tile_pool(name="hf", bufs=4) as hf_pool,
            tc.tile_pool(name="l1", bufs=2) as l1_pool,
            tc.tile_pool(name="tree", bufs=1) as tree_pool,
            tc.tile_pool(name="wm", bufs=2) as wm_pool,
        ):
            wt_sb = wt_pool.tile([D, COLS], BF16)
            xt_sb = xt_pool.tile([D, B], BF16)
            # xt first so group 0's stationary is ready; W lands in tile order
            # so group 0's tile t can start as soon as slice t is in.
            nc.sync.dma_start(xt_sb[:], xt_d[:])
            for t in range(0, NFT + 1, 2):
                c0 = t * TILE
                c1 = min(COLS, c0 + 2 * TILE)
                nc.sync.dma_start(wt_sb[:, c0:c1], wt_d[:, c0:c1])

            # Software-pipelined: group g-1's output DMA is issued in the
            # middle of group g so it never blocks the drain chain.
            NG = B // 128
            # Last group's l1 is split into 6 tiles so its output DMA can
            # start before the group finishes (no long drain at the end).
            SEGW = 8 * 512
            SEGB = list(range(0, L1W, SEGW))
            pending = None  # (l1_tile, group) of the previous group
            for g in range(NG):
                if g < NG - 1:
                    l1 = l1_pool.tile([128, L1W], FP8)
                    segs = [(l1, 0)]
                else:
                    segs = []
                    for i, b0 in enumerate(SEGB):
                        w = min(SEGW, L1W - b0)
                        st = l1_pool.tile([128, w], FP8, tag=f"l1s{i}")
                        segs.append((st, b0))

                def lslice(o0, o1):
                    for seg, base in reversed(segs):
                        if o0 >= base:
                            return seg[:, o0 - base:o1 - base]
                    raise AssertionError

                for t in range(NFT + 1):
                    ps = psum_pool.tile([128, TILE], F32)
                    if t < NFT:
                        for m in range(TILE // MMW):
                            c0 = t * TILE + m * MMW
                            nc.tensor.matmul(
                                ps[:, m * MMW:(m + 1) * MMW],
                                xt_sb[:, g * 128:(g + 1) * 128],
                                wt_sb[:, c0:c0 + MMW],
                                start=True, stop=True,
                            )
                        lo = lslice(t * 512, (t + 1) * 512)
                        if t in FULL_ACT_TILES and g % 2 == 0:
                            h = hf_pool.tile([128, 1024], FP16, tag="hf")
                            nc.scalar.copy(h[:], ps[:])
                            nc.vector.tensor_tensor(
                                lo, h[:, 0:512], h[:, 512:1024], MAX
                            )
                        else:
                            h = hi_pool.tile([128, 512], FP16, tag="hi")
                            nc.scalar.copy(h[:], ps[:, 0:512])
                            nc.vector.tensor_tensor(
                                lo, ps[:, 512:1024], h[:], MAX
                            )
                    else:
                        # tail tile: 1 matmul of 512, split drain
                        nc.tensor.matmul(
                            ps[:, 0:TAIL],
                            xt_sb[:, g * 128:(g + 1) * 128],
                            wt_sb[:, NFT * TILE:COLS],
                            start=True, stop=True,
                        )
                        ht = hi_pool.tile([128, 512], FP16, tag="hi")
                        nc.scalar.copy(ht[:, 0:256], ps[:, 0:256])
                        nc.vector.tensor_tensor(
                            lslice(NFT * 512, L1W), ps[:, 256:512],
                            ht[:, 0:256], MAX,
                        )
                    if pending is not None and t == 8:
                        pl1, pg = pending
                        eng = nc.gpsimd if pg % 2 == 0 else nc.sync
                        eng.dma_start(
                            wmax_d[pg * 128:(pg + 1) * 128, :], pl1[:]
                        )
                        pending = None
                    if (g == NG - 1 and t in (8, 16)
                            and t // 8 <= len(SEGB) - 2):
                        seg, base = segs[t // 8 - 1]
                        nc.gpsimd.dma_start(
                            wmax_d[g * 128:(g + 1) * 128,
                                   base:base + seg.shape[1]],
                            seg[:],
                        )
                if g < NG - 1:
                    pending = (l1, g)
            g = NG - 1
            for i, (seg, base) in enumerate(segs[len(SEGB) - 2:]):
                eng = nc.gpsimd if i % 2 == 0 else nc.sync
                eng.dma_start(
                    wmax_d[g * 128:(g + 1) * 128, base:base + seg.shape[1]],
                    seg[:],
                )
    nc.compile()
    return nc


def _wincols():
    """[NWIN_ALL, WIN] global column per (core, tile, j) window; -1 invalid.

    Full tile t, window j in [0,512): cols t*1024 + j + 512*k, k in {0,1}.
    Tail tile, window j in [0,256):   cols 24576 + j + 256*k,  k in {0,1}.
    """
    global _WINCOLS
    if _WINCOLS is None:
        k = np.arange(WIN)
        full = (
            np.arange(NFT)[:, None, None] * TILE
            + np.arange(NWIN_FT)[None, :, None]
            + 512 * k[None, None, :]
        ).reshape(NFT * NWIN_FT, WIN)
        tail = (
            NFT * TILE + np.arange(NWIN_TAIL)[:, None] + 256 * k[None, :]
        )
        local = np.concatenate([full, tail], axis=0)  # [NWIN, WIN]
        cols = (
            np.arange(NCORES)[:, None, None] * VSHARD + local[None]
        ).reshape(NWIN_ALL, WIN)
        invalid = np.broadcast_to(
            local[None] >= VSHARD, (NCORES, NWIN, WIN)
        ).reshape(NWIN_ALL, WIN)
        cols = cols.copy()
        cols[invalid] = -1
        _WINCOLS = cols.astype(np.int64)
    return _WINCOLS


def _merge(x64, W64, stored, Bw):
    """Exact top-K from device window maxima.  Returns (idx, eps, bad_rows)."""
    wincols = _wincols()
    vals = stored  # [B, NWIN_ALL] f32
    thr = np.partition(vals, NWIN_ALL - TOPK, axis=1)[:, NWIN_ALL - TOPK]
    sel = vals >= (thr[:, None] - Bw)

    out = np.empty((B, TOPK), dtype=np.int64)
    eps = 0.0
    bad_rows = []
    STEP = 64
    for r0 in range(0, B, STEP):
        r1 = r0 + STEP
        sblk = sel[r0:r1]
        maxw = int(sblk.sum(axis=1).max())
        wid = np.full((STEP, maxw), -1, dtype=np.int64)
        for i in range(STEP):
            w = np.flatnonzero(sblk[i])
            wid[i, :len(w)] = w
        cols = np.where(
            wid[:, :, None] >= 0, wincols[wid], -1
        ).reshape(STEP, maxw * WIN)
        valid = cols >= 0
        gW = W64[np.where(valid, cols, 0)]
        exact = np.einsum("bjd,bd->bj", gW, x64[r0:r1])
        exact[~valid] = -np.inf

        ew = exact.reshape(STEP, maxw, WIN).max(axis=2)
        wv = wid >= 0
        dv = np.take_along_axis(vals[r0:r1], np.clip(wid, 0, None), axis=1)
        fin = wv & np.isfinite(ew)
        if fin.any():
            eps = max(eps, float(np.abs(np.where(fin, dv - ew, 0.0)).max()))

        order = np.lexsort((np.where(valid, cols, 2**62), -exact), axis=1)
        top = order[:, :TOPK]
        tv = np.take_along_axis(exact, top, axis=1)
        if not np.isfinite(tv).all():
            bad_rows.extend(r0 + np.flatnonzero(~np.isfinite(tv).all(axis=1)))
        out[r0:r1] = np.take_along_axis(cols, top, axis=1)
    return out, eps, bad_rows


def kernel(x: np.ndarray, W: np.ndarray, topk) -> np.ndarray:
    global LAST_RESULTS, _CACHED_NC
    import os

    import ml_dtypes

    from concourse.bass_utils import run_bass_kernel_spmd

    assert x.shape == (B, D) and W.shape == (VOCAB, D)
    assert int(topk) == TOPK
    x = np.ascontiguousarray(np.asarray(x, dtype=np.float32))
    W = np.ascontiguousarray(np.asarray(W, dtype=np.float32))

    if _CACHED_NC is None:
        _CACHED_NC = build_kernel()
    nc = _CACHED_NC

    xt = np.ascontiguousarray(x.T).astype(ml_dtypes.bfloat16)
    in_maps = []
    for i in range(NCORES):
        sh = np.zeros((D, COLS), dtype=ml_dtypes.bfloat16)
        sh[:, :VSHARD] = W[i * VSHARD:(i + 1) * VSHARD].T.astype(
            ml_dtypes.bfloat16
        )
        in_maps.append({"wt": sh, "xt": xt})

    LAST_RESULTS = run_bass_kernel_spmd(
        nc,
        in_maps,
        core_ids=list(range(NCORES)),
        trace=bool(int(os.environ.get("KERNEL_TRACE", "0"))),
    )
    results = LAST_RESULTS.results

    stored = np.concatenate(
        [np.asarray(results[i]["out_wmax"])
         .view(ml_dtypes.float8_e4m3fn).astype(np.float32)
         for i in range(NCORES)],
        axis=1,
    )  # [B, NWIN_ALL]

    x64 = x.astype(np.float64)
    W64 = W.astype(np.float64)

    Bw = B_SLACK
    for _ in range(3):
        out, eps, bad_rows = _merge(x64, W64, stored, Bw)
        if 2.0 * eps + 0.15 <= Bw and not bad_rows:
            break
        Bw = max(2.0 * (2.0 * eps + 0.15), 2.0 * Bw)
    else:
        bad_rows = list(range(B))

    for r in set(int(r) for r in bad_rows):
        s = x64[r] @ W64.T
        out[r] = np.lexsort((np.arange(VOCAB), -s))[:TOPK]

    return out.astype(np.int32)


# revision 28
# speedup vs baseline: 1.1798x; 1.0012x over previous
"""Distributed exact inner-product top-k (brute-force kNN) on 8 TRN2 NeuronCores.

Sharding: codebook W is split row-wise into 8 shards of 25000 (one per core,
padded to 25088 = 24 tiles x 1024 + 512 with zero columns); x is replicated.
Host pre-transposes both so the contraction dim (128) lands on SBUF partitions.

Device kernel (SPMD, no collectives), per 128-row group and 1024-col tile:
  - 2 x 512-wide bf16 matmuls -> PSUM f32 scores (512 = ISA cap per matmul;
    4-deep PSUM tile pipeline hides the drain-chain sync latency)
  - the tile is drained half by ACT (fp16 copy of the lo half, issued right
    after the first matmul) and half by DVE (tensor_tensor max pairing the
    PSUM hi half against that copy -- two PSUM operands are illegal, and
    this one pass is both the PSUM drain and the pair-reduction); the even
    split keeps ACT (0.833ns/elem) and DVE (1.04ns/elem on a PSUM operand)
    within ~5% of each other, and both run ~95% busy
  - the resulting window maxima (window = 2 cols: {c, c+512} within the
    tile) go straight to DRAM as [1024, 12544] fp8e4 per core, on the idle
    gpsimd engine's DGE ring (alternating with sync) so output transfers
    never queue behind each other (fp8 halves output DMA; the coarse
    storage quantization only widens the host recompute set).  There is no max8/find_index8 (1
    elem/cycle + a second full scan dominated the original kernel) and no
    deeper on-device reduction (DMA bandwidth is cheaper than DVE cycles
    at this balance point).

Host merge: per row, t_hat = 128th-largest stored window max; every window
with stored >= t_hat - B is recomputed exactly in f64.  A window hiding a
true top-128 element necessarily has stored max >= t_hat - 2*eps where eps
bounds |stored - exact| (bf16 matmul noise < 0.25 + fp8e4 storage rounding
<= 4 at these magnitudes); B = 10 covers it with margin.  The measured eps is verified
against the recomputed windows every run; if it nears B/2 the selection is
redone with a wider B from the same stored values, and any bad row falls
back to full exact recompute.  Final top-128 ordered like jax.lax.top_k
(value desc, index asc; ~12 of 131072 entries differ from the reference
where f32 score ties rank differently -- same as the previous kernel).
"""

import numpy as np

B = 1024
D = 128
VOCAB = 200000
NCORES = 8
VSHARD = VOCAB // NCORES   # 25000
NFT = 24                   # full 1024-col tiles per core
TILE = 1024
TAIL = 512                 # tail tile cols
COLS = NFT * TILE + TAIL   # 25088 (padded shard width)
WIN = 2                    # columns per window
NWIN_FT = 512              # windows per full tile
NWIN_TAIL = 256
NWIN = NFT * NWIN_FT + NWIN_TAIL      # 12544 windows per core per row
NWIN_ALL = NCORES * NWIN              # 100352 windows per row
TOPK = 128
MMW = 512

# Tiles whose PSUM is fully drained by ACT (DVE then pair-maxes in fp16)
# instead of the even ACT/DVE split.  Empirically the plain even split
# (empty set) balances best; kept as a tuning knob.
FULL_ACT_TILES = frozenset()

# |stored fp8 window max - exact f64 window max|: bf16 matmul noise
# (measured < 0.2 on this data) + fp8e4 storage rounding (<= 4 for values
# up to 128).  B must exceed twice that; verified against the recomputed
# windows every run and widened automatically if ever violated.
B_SLACK = 10.0

LAST_RESULTS = None  # BassKernelResults of the most recent run (for profiling)
_CACHED_NC = None
_WINCOLS = None      # [NWIN_ALL, WIN] int64 global col per window, -1 invalid


def build_kernel():
    import concourse.bass as bass  # noqa: F401
    import concourse.tile as tile
    from concourse import bacc, mybir

    F32 = mybir.dt.float32
    BF16 = mybir.dt.bfloat16
    FP16 = mybir.dt.float16
    FP8 = mybir.dt.float8e4
    MAX = mybir.AluOpType.max

    nc = bacc.Bacc("TRN2", target_bir_lowering=False, debug=False)
    wt_d = nc.dram_tensor("wt", [D, COLS], BF16, kind="ExternalInput")
    xt_d = nc.dram_tensor("xt", [D, B], BF16, kind="ExternalInput")
    wmax_d = nc.dram_tensor("out_wmax", [B, NWIN], FP8, kind="ExternalOutput")

    L1W = NFT * 512 + TAIL // 2  # 12544

    with tile.TileContext(nc) as tc:
        with (
            tc.tile_pool(name="wt", bufs=1) as wt_pool,
            tc.tile_pool(name="xt", bufs=1) as xt_pool,
            tc.tile_pool(name="psum", bufs=4, space="PSUM") as psum_pool,
            tc.tile_pool(name="hi", bufs=6) as hi_pool,
            tc.tile_pool(name="hf", bufs=4) as hf_pool,
            tc.tile_pool(name="l1", bufs=2) as l1_pool,
            tc.tile_pool(name="tree", bufs=1) as tree_pool,
            tc.tile_pool(name="wm", bufs=2) as wm_pool,
        ):
            wt_sb = wt_pool.tile([D, COLS], BF16)
            xt_sb = xt_pool.tile([D, B], BF16)
            # xt first so group 0's stationary is ready; W lands in tile order
            # so group 0's tile t can start as soon as slice t is in.
            nc.sync.dma_start(xt_sb[:], xt_d[:])
            for t in range(0, NFT + 1, 2):
                c0 = t * TILE
                c1 = min(COLS, c0 + 2 * TILE)
                nc.sync.dma_start(wt_sb[:, c0:c1], wt_d[:, c0:c1])

            # Software-pipelined: group g-1's output DMA is issued in the
            # middle of group g so it never blocks the drain chain.
            NG = B // 128
            # Last group's l1 is split into 6 tiles so its output DMA can
            # start before the group finishes (no long drain at the end).
            SEGW = 8 * 512
            SEGB = list(range(0, L1W, SEGW))
            pending = None  # (l1_tile, group) of the previous group
            for g in range(NG):
                if g < NG - 1:
                    l1 = l1_pool.tile([128, L1W], FP8)
                    segs = [(l1, 0)]
                else:
                    segs = []
                    for i, b0 in enumerate(SEGB):
                        w = min(SEGW, L1W - b0)
                        st = l1_pool.tile([128, w], FP8, tag=f"l1s{i}")
                        segs.append((st, b0))

                def lslice(o0, o1):
                    for seg, base in reversed(segs):
                        if o0 >= base:
                            return seg[:, o0 - base:o1 - base]
                    raise AssertionError

                for t in range(NFT + 1):
                    ps = psum_pool.tile([128, TILE], F32)
                    if t < NFT:
                        for m in range(TILE // MMW):
                            c0 = t * TILE + m * MMW
                            nc.tensor.matmul(
                                ps[:, m * MMW:(m + 1) * MMW],
                                xt_sb[:, g * 128:(g + 1) * 128],
                                wt_sb[:, c0:c0 + MMW],
                                start=True, stop=True,
                            )
                        lo = lslice(t * 512, (t + 1) * 512)
                        if t in FULL_ACT_TILES and g % 2 == 0:
                            h = hf_pool.tile([128, 1024], FP16, tag="hf")
                            nc.scalar.copy(h[:], ps[:])
                            nc.vector.tensor_tensor(
                                lo, h[:, 0:512], h[:, 512:1024], MAX
                            )
                        else:
                            h = hi_pool.tile([128, 512], FP16, tag="hi")
                            nc.scalar.copy(h[:], ps[:, 0:512])
                            nc.vector.tensor_tensor(
                                lo, ps[:, 512:1024], h[:], MAX
                            )
                    else:
                        # tail tile: 1 matmul of 512, split drain
                        nc.tensor.matmul(
                            ps[:, 0:TAIL],
                            xt_sb[:, g * 128:(g + 1) * 128],
                            wt_sb[:, NFT * TILE:COLS],
                            start=True, stop=True,
                        )
                        ht = hi_pool.tile([128, 512], FP16, tag="hi")
                        nc.scalar.copy(ht[:, 0:256], ps[:, 0:256])
                        nc.vector.tensor_tensor(
                            lslice(NFT * 512, L1W), ps[:, 256:512],
                            ht[:, 0:256], MAX,
                        )
                    if pending is not None and t == 8:
                        pl1, pg = pending
                        eng = nc.gpsimd if pg % 2 == 0 else nc.sync
                        eng.dma_start(
                            wmax_d[pg * 128:(pg + 1) * 128, :], pl1[:]
                        )
                        pending = None
                    if (g == NG - 1 and t in (8, 16)
                            and t // 8 <= len(SEGB) - 2):
                        seg, base = segs[t // 8 - 1]
                        nc.gpsimd.dma_start(
                            wmax_d[g * 128:(g + 1) * 128,
                                   base:base + seg.shape[1]],
                            seg[:],
                        )
                if g < NG - 1:
                    pending = (l1, g)
            g = NG - 1
            for i, (seg, base) in enumerate(segs[len(SEGB) - 2:]):
                eng = nc.gpsimd if i % 2 == 0 else nc.sync
                eng.dma_start(
                    wmax_d[g * 128:(g + 1) * 128, base:base + seg.shape[1]],
                    seg[:],
                )
    nc.compile()
    return nc


def _wincols():
    """[NWIN_ALL, WIN] global column per (core, tile, j) window; -1 invalid.

    Full tile t, window j in [0,512): cols t*1024 + j + 512*k, k in {0,1}.
    Tail tile, window j in [0,256):   cols 24576 + j + 256*k,  k in {0,1}.
    """
    global _WINCOLS
    if _WINCOLS is None:
        k = np.arange(WIN)
        full = (
            np.arange(NFT)[:, None, None] * TILE
            + np.arange(NWIN_FT)[None, :, None]
            + 512 * k[None, None, :]
        ).reshape(NFT * NWIN_FT, WIN)
        tail = (
            NFT * TILE + np.arange(NWIN_TAIL)[:, None] + 256 * k[None, :]
        )
        local = np.concatenate([full, tail], axis=0)  # [NWIN, WIN]
        cols = (
            np.arange(NCORES)[:, None, None] * VSHARD + local[None]
        ).reshape(NWIN_ALL, WIN)
        invalid = np.broadcast_to(
            local[None] >= VSHARD, (NCORES, NWIN, WIN)
        ).reshape(NWIN_ALL, WIN)
        cols = cols.copy()
        cols[invalid] = -1
        _WINCOLS = cols.astype(np.int64)
    return _WINCOLS


def _merge(x64, W64, stored, Bw):
    """Exact top-K from device window maxima.  Returns (idx, eps, bad_rows)."""
    wincols = _wincols()
    vals = stored  # [B, NWIN_ALL] f32
    thr = np.partition(vals, NWIN_ALL - TOPK, axis=1)[:, NWIN_ALL - TOPK]
    sel = vals >= (thr[:, None] - Bw)

    out = np.empty((B, TOPK), dtype=np.int64)
    eps = 0.0
    bad_rows = []
    STEP = 64
    for r0 in range(0, B, STEP):
        r1 = r0 + STEP
        sblk = sel[r0:r1]
        maxw = int(sblk.sum(axis=1).max())
        wid = np.full((STEP, maxw), -1, dtype=np.int64)
        for i in range(STEP):
            w = np.flatnonzero(sblk[i])
            wid[i, :len(w)] = w
        cols = np.where(
            wid[:, :, None] >= 0, wincols[wid], -1
        ).reshape(STEP, maxw * WIN)
        valid = cols >= 0
        gW = W64[np.where(valid, cols, 0)]
        exact = np.einsum("bjd,bd->bj", gW, x64[r0:r1])
        exact[~valid] = -np.inf

        ew = exact.reshape(STEP, maxw, WIN).max(axis=2)
        wv = wid >= 0
        dv = np.take_along_axis(vals[r0:r1], np.clip(wid, 0, None), axis=1)
        fin = wv & np.isfinite(ew)
        if fin.any():
            eps = max(eps, float(np.abs(np.where(fin, dv - ew, 0.0)).max()))

        order = np.lexsort((np.where(valid, cols, 2**62), -exact), axis=1)
        top = order[:, :TOPK]
        tv = np.take_along_axis(exact, top, axis=1)
        if not np.isfinite(tv).all():
            bad_rows.extend(r0 + np.flatnonzero(~np.isfinite(tv).all(axis=1)))
        out[r0:r1] = np.take_along_axis(cols, top, axis=1)
    return out, eps, bad_rows


def kernel(x: np.ndarray, W: np.ndarray, topk) -> np.ndarray:
    global LAST_RESULTS, _CACHED_NC
    import os

    import ml_dtypes

    from concourse.bass_utils import run_bass_kernel_spmd

    assert x.shape == (B, D) and W.shape == (VOCAB, D)
    assert int(topk) == TOPK
    x = np.ascontiguousarray(np.asarray(x, dtype=np.float32))
    W = np.ascontiguousarray(np.asarray(W, dtype=np.float32))

    if _CACHED_NC is None:
        _CACHED_NC = build_kernel()
    nc = _CACHED_NC

    xt = np.ascontiguousarray(x.T).astype(ml_dtypes.bfloat16)
    in_maps = []
    for i in range(NCORES):
        sh = np.zeros((D, COLS), dtype=ml_dtypes.bfloat16)
        sh[:, :VSHARD] = W[i * VSHARD:(i + 1) * VSHARD].T.astype(
            ml_dtypes.bfloat16
        )
        in_maps.append({"wt": sh, "xt": xt})

    LAST_RESULTS = run_bass_kernel_spmd(
        nc,
        in_maps,
        core_ids=list(range(NCORES)),
        trace=bool(int(os.environ.get("KERNEL_TRACE", "0"))),
    )
    results = LAST_RESULTS.results

    stored = np.concatenate(
        [np.asarray(results[i]["out_wmax"])
         .view(ml_dtypes.float8_e4m3fn).astype(np.float32)
         for i in range(NCORES)],
        axis=1,
    )  # [B, NWIN_ALL]

    x64 = x.astype(np.float64)
    W64 = W.astype(np.float64)

    Bw = B_SLACK
    for _ in range(3):
        out, eps, bad_rows = _merge(x64, W64, stored, Bw)
        if 2.0 * eps + 0.15 <= Bw and not bad_rows:
            break
        Bw = max(2.0 * (2.0 * eps + 0.15), 2.0 * Bw)
    else:
        bad_rows = list(range(B))

    for r in set(int(r) for r in bad_rows):
        s = x64[r] @ W64.T
        out[r] = np.lexsort((np.arange(VOCAB), -s))[:TOPK]

    return out.astype(np.int32)
